# revision 22
# baseline (speedup 1.0000x reference)
"""CFNO kernel for Trainium2 (8 NeuronCores, data-parallel over batch).

Math: the reference's FFT -> ComplexLinear -> Re(IFFT) chain is linear in the
patch vector p[n, 256], so it collapses to y = p @ M.T + cvec with
M = Re(G @ (W_r + i W_i) @ F)  (F = 256-pt DFT matrix, G = 16-pt IDFT/16).
That makes the whole front end a stride-16 16x16-patch conv with 16 output
channels, computed as accumulating K=128 float32r matmuls with
block-diagonal weights (no im2col, no transposes).

Per-core layout: patch-row i = 8*w + il (w = window 0..15, il = 0..7).
Image rows r = 16*i + s1 = 128*w + (16*il + s1): window w is 128 contiguous
image rows, SBUF partition = (il, s1) = natural row order.  Stage-1 matmul
contracts (il, s1) with lhsT[(il,s1), (d,il')] = delta(il,il') * M[d,s1,s2],
accumulated over s2 (rhs free-slices columns c = 16j + s2).  Output
y[(d,il), (w, j)] with w and j on the free axis.

Depthwise 3x3 conv: j shifts are free-axis AP offsets (zero halo columns);
i shifts are il +/- 1 partition shifts expressed in banded lhsT matrices,
with "wrap" matmuls for the il = 7 <-> 0 carries (w +/- 1 on free).  Conv
banks are interleaved with stage-1 groups so they hide under the input DMA
stream.  BatchNorm: per-partition bn_stats, partition-reduce via a delta
matmul, 128-byte cross-core AllReduce, broadcast back via a second delta
matmul, final per-partition affine on the Scalar engine.
"""

import os
from contextlib import ExitStack

import numpy as np

import concourse.bass as bass
import concourse.mybir as mybir
import concourse.tile as tile
from concourse import bacc, bass_utils
from concourse.bass_interp import get_hw_module

F32 = mybir.dt.float32
F32R = mybir.dt.float32r
AF = mybir.ActivationFunctionType
OP = mybir.AluOpType
NCORES = 8
D = 16
EPS = 1e-5

# interior taps, (0,0) first so it initializes every element of each bank
_TAPS = [(0, 0)] + [
    (di, dj) for di in (-1, 0, 1) for dj in (-1, 0, 1) if (di, dj) != (0, 0)
]


def _tap_index(di, dj):
    return (di + 1) * 3 + (dj + 1)


def _conv_jobs_for_bank(bk):
    """(tap_idx, out_w0, out_w1_incl, in_w0, dj) jobs for psum bank bk."""
    jobs = []
    for di, dj in _TAPS:
        jobs.append((_tap_index(di, dj), 4 * bk, 4 * bk + 3, 4 * bk, dj))
    for di in (1, -1):
        for dj in (-1, 0, 1):
            t = (9 if di == 1 else 12) + (dj + 1)
            w_lo, w_hi = (0, 14) if di == 1 else (1, 15)
            r0 = max(4 * bk, w_lo)
            r1 = min(4 * bk + 3, w_hi)
            if r0 <= r1:
                jobs.append((t, r0, r1, r0 + di, dj))
    return jobs


def _build_program(collective=True, upto="full"):
    # upto: "dma" | "s1" | "conv" | "full" — truncated variants for profiling
    ndev = NCORES if collective else 1
    nc = bacc.Bacc("TRN2", target_bir_lowering=False, debug=False, num_devices=ndev)

    x_d = nc.dram_tensor("x", [2048, 2048], F32, kind="ExternalInput")
    ws_d = nc.dram_tensor("wstack", [16, 128, 128], F32, kind="ExternalInput")
    cw_d = nc.dram_tensor("convw", [15, 128, 128], F32, kind="ExternalInput")
    dlt_d = nc.dram_tensor("deltaT", [128, 16], F32, kind="ExternalInput")
    bct_d = nc.dram_tensor("bcastT", [16, 128], F32, kind="ExternalInput")
    cvb_d = nc.dram_tensor("cvecb", [128, 1], F32, kind="ExternalInput")
    gb_d = nc.dram_tensor("gb16", [16, 2], F32, kind="ExternalInput")
    # raw device layout [p=(d,il), (w, j)]; host unshard permutes to [d,i,j]
    out_d = nc.dram_tensor("out", [128, 2048], F32, kind="ExternalOutput")

    with tile.TileContext(nc) as tc, ExitStack() as ctx:
        consts = ctx.enter_context(tc.tile_pool(name="consts", bufs=1))
        xpool = ctx.enter_context(tc.tile_pool(name="xpool", bufs=2))
        ysb_p = ctx.enter_context(tc.tile_pool(name="ysb", bufs=1))
        csb_p = ctx.enter_context(tc.tile_pool(name="csb", bufs=1))
        small = ctx.enter_context(tc.tile_pool(name="small", bufs=1))
        dram = ctx.enter_context(tc.tile_pool(name="dram", bufs=1, space="DRAM"))
        yps_p = ctx.enter_context(tc.tile_pool(name="yps", bufs=2, space="PSUM"))
        cps_p = ctx.enter_context(tc.tile_pool(name="cps", bufs=1, space="PSUM"))
        sps_p = ctx.enter_context(tc.tile_pool(name="sps", bufs=1, space="PSUM"))

        # constant loads ride the Activation HWDGE ring so the SP ring is
        # free for the input stream from cycle zero
        w_sb = consts.tile([128, 16, 128], F32)
        nc.scalar.dma_start(out=w_sb[:], in_=ws_d.ap().rearrange("s k m -> k s m"))
        cw_sb = consts.tile([128, 15, 128], F32)
        nc.scalar.dma_start(out=cw_sb[:], in_=cw_d.ap().rearrange("t k m -> k t m"))
        dlt_sb = consts.tile([128, 16], F32)
        nc.scalar.dma_start(out=dlt_sb[:], in_=dlt_d.ap())
        bct_sb = consts.tile([16, 128], F32)
        nc.scalar.dma_start(out=bct_sb[:], in_=bct_d.ap())
        cvb_sb = consts.tile([128, 1], F32)
        nc.scalar.dma_start(out=cvb_sb[:], in_=cvb_d.ap())
        gb_sb = consts.tile([16, 2], F32)
        nc.scalar.dma_start(out=gb_sb[:], in_=gb_d.ap())
        eps_t = consts.tile([16, 1], F32)
        nc.vector.memset(eps_t[:], float(EPS))

        # y with a zero halo column on each side of j (130 slots per w)
        y_sb = ysb_p.tile([128, 16, 130], F32)
        nc.vector.memset(y_sb[:, :, 0:1], 0.0)
        nc.vector.memset(y_sb[:, :, 129:130], 0.0)

        conv_sb = csb_p.tile([128, 16, 128], F32)
        cp = cps_p.tile([128, 16, 128], F32)  # 4 banks
        stats6 = small.tile([128, 4, 6], F32)

        # image rows r = 512*g + 128*wl + p (p = 16*il + s1), cols c = 16*j+s2
        xv = x_d.ap().rearrange(
            "(g wl p) (j s2) -> g p wl j s2", g=4, wl=4, p=128, s2=16
        )

        xg_last = None

        def emit_s1_group(g):
            nonlocal xg_last
            xg = xpool.tile([128, 4, 128, 16], F32, tag="xg", name=f"xg{g}")
            nc.sync.dma_start(out=xg[:], in_=xv[g])
            xg_last = xg
            if upto == "dma":
                return
            yp = yps_p.tile([128, 4, 128], F32, tag="yp", name=f"yp{g}")
            for s2 in range(16):
                nc.tensor.matmul(
                    yp[:],
                    w_sb[:, s2, :].bitcast(F32R),
                    xg[:, :, :, s2].bitcast(F32R),
                    start=(s2 == 0),
                    stop=(s2 == 15),
                )
            # evict + add patchify bias cvec (per-partition, only d-dep)
            nc.scalar.activation(
                out=y_sb[:, 4 * g : 4 * g + 4, 1:129],
                in_=yp[:],
                func=AF.Identity,
                bias=cvb_sb[:, 0:1],
                scale=1.0,
            )

        def emit_conv_bank(bk):
            jobs = _conv_jobs_for_bank(bk)
            for idx, (t, r0, r1, ri, dj) in enumerate(jobs):
                n_w = r1 - r0 + 1
                nc.tensor.matmul(
                    cp[:, r0 : r1 + 1, :],
                    cw_sb[:, t, :].bitcast(F32R),
                    y_sb[:, ri : ri + n_w, 1 + dj : 129 + dj].bitcast(F32R),
                    start=(idx == 0),
                    stop=(idx == len(jobs) - 1),
                )
            sl = slice(4 * bk, 4 * bk + 4)
            nc.scalar.copy(out=conv_sb[:, sl, :], in_=cp[:, sl, :])
            nc.vector.bn_stats(
                out=stats6[:, bk, :],
                in_=conv_sb[:, sl, :].rearrange("p a b -> p (a b)"),
            )

        # ---- interleaved stage-1 / conv emission ----------------------
        emit_s1_group(0)
        emit_s1_group(1)
        if upto in ("conv", "full"):
            emit_conv_bank(0)
        emit_s1_group(2)
        if upto in ("conv", "full"):
            emit_conv_bank(1)
        emit_s1_group(3)
        if upto in ("conv", "full"):
            emit_conv_bank(2)
            emit_conv_bank(3)

        if upto == "dma":
            nc.sync.dma_start(
                out=out_d.ap(),
                in_=xg_last[:, 0, :, :].rearrange("p a b -> p (a b)"),
            )
        elif upto == "s1":
            nc.sync.dma_start(out=out_d.ap(), in_=y_sb[:, :, 1:129])
        elif upto == "conv":
            nc.sync.dma_start(out=out_d.ap(), in_=conv_sb[:])
        else:
            # ---- BatchNorm stats + AllReduce --------------------------
            mv = small.tile([128, 2], F32)
            nc.vector.bn_aggr(out=mv[:], in_=stats6[:])
            # stats2 = (mean, E[x^2]) per partition
            stats2 = small.tile([128, 2], F32)
            nc.vector.tensor_copy(out=stats2[:, 0:1], in_=mv[:, 0:1])
            nc.vector.scalar_tensor_tensor(
                out=stats2[:, 1:2],
                in0=mv[:, 0:1],
                scalar=mv[:, 0:1],
                in1=mv[:, 1:2],
                op0=OP.mult,
                op1=OP.add,
            )
            # partition-reduce over il (8 partitions per d) via delta matmul
            red_sb = small.tile([16, 2], F32)
            ps16 = sps_p.tile([16, 2], F32, tag="s")
            nc.tensor.matmul(ps16[:], dlt_sb[:], stats2[:], start=True, stop=True)
            nc.scalar.copy(out=red_sb[:], in_=ps16[:])

            bounce_in = dram.tile([16, 2], F32)
            bounce_out = dram.tile([16, 2], F32)
            nc.sync.dma_start(out=bounce_in[:], in_=red_sb[:])
            if collective:
                nc.gpsimd.collective_compute(
                    "AllReduce",
                    mybir.AluOpType.add,
                    ins=[bounce_in.opt()],
                    outs=[bounce_out.opt()],
                    replica_groups=[list(range(NCORES))],
                )
            else:
                nc.sync.dma_start(out=bounce_out[:], in_=bounce_in[:])
            ar_sb = small.tile([16, 2], F32)
            nc.sync.dma_start(out=ar_sb[:], in_=bounce_out[:])

            # scale = gamma * rsqrt(var+eps), bias = beta - mean*scale
            inv_n = 1.0 / (NCORES * 8.0)  # 64 partition-instances per channel
            ar2 = small.tile([16, 2], F32)
            nc.vector.tensor_scalar_mul(ar2[:], ar_sb[:], inv_n)
            q_t = small.tile([16, 1], F32)  # mean^2 - E[x^2] = -var
            nc.vector.scalar_tensor_tensor(
                out=q_t[:],
                in0=ar2[:, 0:1],
                scalar=ar2[:, 0:1],
                in1=ar2[:, 1:2],
                op0=OP.mult,
                op1=OP.subtract,
            )
            sd_t = small.tile([16, 1], F32)
            nc.scalar.activation(
                sd_t[:], q_t[:], AF.Sqrt, bias=eps_t[:], scale=-1.0
            )
            rstd_t = small.tile([16, 1], F32)
            nc.vector.reciprocal(rstd_t[:], sd_t[:])
            sb2 = small.tile([16, 2], F32)
            nc.vector.tensor_mul(sb2[:, 0:1], gb_sb[:, 0:1], rstd_t[:])
            mscale = small.tile([16, 1], F32)
            nc.vector.tensor_mul(mscale[:], ar2[:, 0:1], sb2[:, 0:1])
            nc.vector.tensor_sub(out=sb2[:, 1:2], in0=gb_sb[:, 1:2], in1=mscale[:])

            # broadcast (scale, bias) from 16 d-partitions to all 128
            sbias = small.tile([128, 2], F32)
            psb = sps_p.tile([128, 2], F32, tag="s")
            nc.tensor.matmul(psb[:], bct_sb[:], sb2[:], start=True, stop=True)
            nc.scalar.copy(out=sbias[:], in_=psb[:])

            # final affine + store, in two chunks to overlap ACT with DMA
            out_sb = csb_p.tile([128, 16, 128], F32)
            for h in range(2):
                sl = slice(8 * h, 8 * h + 8)
                nc.scalar.activation(
                    out=out_sb[:, sl, :],
                    in_=conv_sb[:, sl, :],
                    func=AF.Identity,
                    bias=sbias[:, 1:2],
                    scale=sbias[:, 0:1],
                )
                nc.sync.dma_start(
                    out=out_d.ap()[:, 1024 * h : 1024 * h + 1024],
                    in_=out_sb[:, sl, :],
                )

    nc.compile()
    return nc


def _build_consts(W_r, b_r, W_i, b_i, conv_w, gamma, beta):
    feat = 256
    kk = np.arange(feat)
    F = np.exp(-2j * np.pi * np.outer(kk, kk) / feat)  # DFT
    dd = np.arange(D)
    G = np.exp(2j * np.pi * np.outer(dd, dd) / D) / D  # IDFT
    Wc = W_r.astype(np.float64) + 1j * W_i.astype(np.float64)
    bc = (1 + 1j) * (b_r.astype(np.float64) + 1j * b_i.astype(np.float64))
    M = np.real(G @ Wc @ F)  # [16, 256]
    cvec = np.real(G @ bc)  # [16]

    M3 = M.reshape(D, 16, 16)  # [d, s1, s2]
    ws = np.zeros((16, 8, 16, D, 8), np.float64)  # [s2, il, s1, d, il2]
    m_t = M3.transpose(2, 1, 0)  # [s2, s1, d]
    for il in range(8):
        ws[:, il, :, :, il] = m_t
    wstack = ws.reshape(16, 128, 128).astype(np.float32)

    cw = conv_w[:, 0].astype(np.float64)  # [16, 3, 3]
    cwst = np.zeros((15, 128, 128), np.float64)
    # interior taps: lhsT[(d, il+di), (d, il)] = w[d, di+1, dj+1]
    for di in (-1, 0, 1):
        for dj in (-1, 0, 1):
            t = _tap_index(di, dj)
            for d in range(D):
                for il in range(8):
                    il_k = il + di
                    if 0 <= il_k <= 7:
                        cwst[t][d * 8 + il_k, d * 8 + il] = cw[d, di + 1, dj + 1]
    # wrap taps: il 7 <-> 0 carries (w +/- 1 handled by the rhs AP)
    for di in (1, -1):
        for dj in (-1, 0, 1):
            t = (9 if di == 1 else 12) + (dj + 1)
            for d in range(D):
                if di == 1:
                    cwst[t][d * 8 + 0, d * 8 + 7] = cw[d, 2, dj + 1]
                else:
                    cwst[t][d * 8 + 7, d * 8 + 0] = cw[d, 0, dj + 1]
    cwst = cwst.astype(np.float32)

    dlt = np.zeros((128, 16), np.float32)
    dlt[np.arange(128), np.arange(128) // 8] = 1.0
    bct = np.zeros((16, 128), np.float32)
    bct[np.arange(128) // 8, np.arange(128)] = 1.0
    cvb = cvec.astype(np.float32)[np.arange(128) // 8].reshape(128, 1)
    gb16 = np.stack(
        [gamma.astype(np.float32), beta.astype(np.float32)], axis=1
    )  # [16, 2]
    return {
        "wstack": wstack,
        "convw": cwst,
        "deltaT": dlt,
        "bcastT": bct,
        "cvecb": np.ascontiguousarray(cvb),
        "gb16": np.ascontiguousarray(gb16),
    }


_NC_CACHE = []
LAST_RESULT = None


def kernel(x, W_r, b_r, W_i, b_i, conv_w, conv_b, gamma, beta):
    # conv_b is intentionally unused: BatchNorm subtracts the per-channel
    # mean, so a constant per-channel conv bias cancels exactly.
    global LAST_RESULT
    if not _NC_CACHE:
        nc = _build_program()
        nc.m = get_hw_module(nc.m)
        _NC_CACHE.append(nc)
    nc = _NC_CACHE[0]

    consts = _build_consts(W_r, b_r, W_i, b_i, conv_w, gamma, beta)
    x = np.asarray(x, dtype=np.float32)
    in_maps = []
    for c in range(NCORES):
        m = {"x": np.ascontiguousarray(x[c, 0])}
        m.update(consts)
        in_maps.append(m)

    trace = bool(int(os.environ.get("KERNEL_TRACE", "0")))
    try:
        res = bass_utils.run_bass_kernel_spmd(
            nc, in_maps, core_ids=list(range(NCORES)), trace=trace
        )
    except ModuleNotFoundError:
        # axon NTFF profiling hook unavailable in this container
        res = bass_utils.run_bass_kernel_spmd(
            nc, in_maps, core_ids=list(range(NCORES)), trace=False
        )
    LAST_RESULT = res
    # device layout [p=(d,il), (w,j)] -> [d, i=8w+il, j]
    out = np.stack(
        [
            res.results[c]["out"]
            .reshape(D, 8, 16, 128)
            .transpose(0, 2, 1, 3)
            .reshape(D, 128, 128)
            for c in range(NCORES)
        ],
        axis=0,
    )
    return np.ascontiguousarray(out, dtype=np.float32)
